# revision 21
# baseline (speedup 1.0000x reference)
"""Sliding-window causal self-attention (B=2, T=2048, D=1024, H=16, dk=64, W=512)
on 8 Trainium2 NeuronCores.

Sharding: core = (b, hg) for b in {0,1}, head-group hg in {0..3}.
Data parallel over batch, tensor parallel over heads: each core gets
x[b]^T, the 4-head column slices of Wq/Wk/Wv (+bq slice) and the matching
row slice of Wo, and produces a partial [T, D] output (fp16).  Host gathers
with out[b] = sum_hg partial[b,hg] + (bv @ Wo + bo) in fp32.

Math notes (exact softmax identities, validated vs reference):
 - bk shifts every logit of a row by a per-row constant -> cancels in softmax.
 - bv enters the output linearly with weights summing to 1 -> folded into the
   host-side bias term bv @ Wo (+ bo), added once after the cross-core sum.
 - no max-subtraction in softmax: logits are O(1), fp16 exp is safe
   (|S/8| < 6 -> exp < 403 << 65504).

All matmul operands are float16: 1 cycle/column at ANY moving width on the
PE, half the SBUF bandwidth, half the LDWEIGHTS time, half the DMA.
PSUM accumulation stays fp32.  ~5e-4 relative error end to end.

Schedule (v3): both head-pairs' attention J-loops run interleaved in one
18-step pipeline (pair23 lags pair01 by 2 steps), with the Q/K/V projection
work chopped into ~2-4k-cycle units and woven between the S-matmul blocks so
the PE never starves while the ACT (exp) pipeline drains.  This also spreads
the exp/normalize/output-projection load evenly across the whole kernel
instead of concentrating it in a second-half phase.

Step s: S(pair01, J=s) | pre-filler | S(pair23, J=s-2) | post-filler+groups.
PV groups: pair01 at s=4g+3, pair23 at s=4g+5; the output projection for
group g runs right after pair23's group g normalizes (all 4 heads ready).

The V_aug stationary carries a 64-wide ones block ahead of the 64 V
columns, so the PV matmul emits the softmax denominator already broadcast
across psum partitions 0:64 -- no denominator copy and no rank-1 broadcast
matmul (and the custom-DVE reciprocal reads psum at base partition 0; it
returns garbage on hardware at base 64).  Engine budget: PE ~85us of
matmul streaming; ACT = exp (64x ~0.68us) + q-bias + stage copies; DVE =
K/V copies, reciprocal, osb muls; GPSIMD = mask multiplies.  Input DMAs
are plain contiguous per-chunk transfers (a rearranged multi-descriptor
DMA's completion semaphore was observed to fire before all bytes landed,
corrupting first-run results).  Measured: ~130us HW exec (baseline 183us),
~5.6e-4 relative error.
"""

import math
from contextlib import ExitStack

import numpy as np

import concourse.bass as bass
import concourse.mybir as mybir
import concourse.tile as tile
from concourse import bacc
from concourse.bass_utils import run_bass_kernel_spmd

F32 = mybir.dt.float32
F16 = mybir.dt.float16

T = 2048
D = 1024
NHEAD = 16
DK = 64
WINDOW = 512
HPC = 4            # heads per core
HCOLS = HPC * DK   # 256 projected columns per core
NJ = T // 128      # 16 j/query blocks
NKC = D // 128     # 8 contraction chunks over D
NG = 4             # query-block groups of 512

_NC_CACHE = {}


def _emit(tc):
    nc = tc.nc
    xT_d = nc.dram_tensor("xT", [D, T], F16, kind="ExternalInput").ap()
    wq_d = nc.dram_tensor("wq", [D, HCOLS], F16, kind="ExternalInput").ap()
    wk_d = nc.dram_tensor("wk", [D, HCOLS], F16, kind="ExternalInput").ap()
    wv_d = nc.dram_tensor("wv", [D, HCOLS], F16, kind="ExternalInput").ap()
    wo_d = nc.dram_tensor("wo", [HCOLS, D], F16, kind="ExternalInput").ap()
    bq_d = nc.dram_tensor("bqp", [128, 2], F32, kind="ExternalInput").ap()
    mlo_d = nc.dram_tensor("mlo", [128, 128], F16, kind="ExternalInput").ap()
    mhi_d = nc.dram_tensor("mhi", [128, 128], F16, kind="ExternalInput").ap()
    out_d = nc.dram_tensor("out", [T, D], F16, kind="ExternalOutput").ap()

    with ExitStack() as ctx:
        const_pool = ctx.enter_context(tc.tile_pool(name="const", bufs=1))
        qk_pool = ctx.enter_context(tc.tile_pool(name="qk", bufs=1))
        w_pool = ctx.enter_context(tc.tile_pool(name="w", bufs=1))
        xt_pool = ctx.enter_context(tc.tile_pool(name="xt", bufs=3))
        pt_pool = ctx.enter_context(tc.tile_pool(name="pt", bufs=36))
        nrm_pool = ctx.enter_context(tc.tile_pool(name="nrm", bufs=4))
        stage_pool = ctx.enter_context(tc.tile_pool(name="stage", bufs=2))
        ps_s = ctx.enter_context(tc.tile_pool(name="ps_s", bufs=2, space="PSUM"))
        ps_pv = ctx.enter_context(tc.tile_pool(name="ps_pv", bufs=2, space="PSUM"))
        ps_mi = ctx.enter_context(tc.tile_pool(name="ps_mi", bufs=2, space="PSUM"))

        bq_sb = const_pool.tile([128, 2], F32)
        mask_lo = const_pool.tile([128, 128], F16)   # keep c >= p (upper incl)
        mask_hi = const_pool.tile([128, 128], F16)   # keep c < p (strict lower)

        wo_sb = qk_pool.tile([128, 2, D], F16)
        # V storage [j-part, J, head, 2*dk]; cols 0:64 of each head slot
        # are 1.0, so the PV matmul emits the softmax denominator already
        # broadcast across psum partitions 0:64 (base 0: the custom-DVE
        # reciprocal mishandles a non-zero base partition on hardware).
        v_sb = qk_pool.tile([128, NJ, HPC, 2 * DK], F16)
        q_sb = qk_pool.tile([128, 2, T], F16)
        k_sb = qk_pool.tile([128, 2, T], F16)
        osb = qk_pool.tile([128, 2, T], F16)   # normalized O^T

        wq_sb = w_pool.tile([128, NKC, HCOLS], F16)
        wk_sb = w_pool.tile([128, NKC, HCOLS], F16)
        wv_sb = w_pool.tile([128, NKC, HCOLS], F16)

        # ---- x^T streamed by 512-column blocks, 3 SBUF slots ----
        xt_tiles = {}

        def xt_dma(cb, engs=(None,)):
            xt_tiles[cb] = xt_pool.tile([128, NKC, 512], F16, tag="xt",
                                        name=f"xt_c{cb}")
            for k in range(NKC):
                eng = engs[k % len(engs)] or nc.sync
                eng.dma_start(
                    xt_tiles[cb][:, k, :],
                    xT_d[k * 128:(k + 1) * 128, cb * 512:(cb + 1) * 512])

        # prologue DMAs, ordered by first use; plain contiguous chunks only.
        # sync carries the critical path; scalar/vector are idle here and
        # absorb half the issue cost.
        # first-need-first: u_q(0,0) consumes (wq[k], xt0[k]) pairs in k
        # order, so issue each pair back to back on separate queues.
        xt_tiles[0] = xt_pool.tile([128, NKC, 512], F16, tag="xt",
                                   name="xt_c0")
        for k in range(NKC):
            weng = nc.scalar if k % 2 == 0 else nc.gpsimd
            weng.dma_start(wq_sb[:, k, :], wq_d[k * 128:(k + 1) * 128, :])
            nc.sync.dma_start(
                xt_tiles[0][:, k, :], xT_d[k * 128:(k + 1) * 128, 0:512])
        nc.gpsimd.dma_start(bq_sb[:], bq_d[:, :])
        for k in range(NKC):
            eng = nc.sync if k % 2 == 0 else nc.scalar
            eng.dma_start(wk_sb[:, k, :], wk_d[k * 128:(k + 1) * 128, :])
        nc.gpsimd.dma_start(mask_lo[:], mlo_d[:, :])
        nc.gpsimd.dma_start(mask_hi[:], mhi_d[:, :])
        xt_dma(1, engs=(nc.sync, nc.scalar))
        for k in range(NKC):
            eng = nc.gpsimd if k % 2 == 0 else nc.scalar
            eng.dma_start(wv_sb[:, k, :], wv_d[k * 128:(k + 1) * 128, :])
        for c in range(2):
            nc.sync.dma_start(wo_sb[:, c, :], wo_d[c * 128:(c + 1) * 128, :])
        xt_dma(2, engs=(nc.sync, nc.gpsimd))
        # ones block of V_aug via memsets on prologue-idle engines
        nc.gpsimd.memset(v_sb[:, 0:NJ // 2, :, 0:DK], 1.0)
        nc.vector.memset(v_sb[:, NJ // 2:NJ, :, 0:DK], 1.0)

        # ---- projection units (~2-4k PE cycles each) ----
        def u_q(cb, m):
            xt = xt_tiles[cb]
            qp = ps_mi.tile([128, 512], F32, tag="mi", name=f"qp{cb}{m}")
            for k in range(NKC):
                nc.tensor.matmul(
                    qp[:], wq_sb[:, k, m * 128:(m + 1) * 128],
                    xt[:, k, :], start=(k == 0), stop=(k == NKC - 1),
                )
            nc.scalar.activation(
                q_sb[:, m, cb * 512:(cb + 1) * 512], qp[:],
                mybir.ActivationFunctionType.Identity,
                bias=bq_sb[:, m:m + 1],
            )

        def u_k(cb, m):
            xt = xt_tiles[cb]
            kp = ps_mi.tile([128, 512], F32, tag="mi", name=f"kp{cb}{m}")
            for k in range(NKC):
                nc.tensor.matmul(
                    kp[:], wk_sb[:, k, m * 128:(m + 1) * 128],
                    xt[:, k, :], start=(k == 0), stop=(k == NKC - 1),
                )
            nc.vector.tensor_copy(k_sb[:, m, cb * 512:(cb + 1) * 512], kp[:])

        def u_v(r):
            cb = r // 4
            xt = xt_tiles[cb]
            vp = ps_mi.tile([128, HPC, DK], F32, tag="mi", name=f"vp{r}")
            for k in range(NKC):
                nc.tensor.matmul(
                    vp[:], xt[:, k, (r % 4) * 128:(r % 4) * 128 + 128],
                    wv_sb[:, k, :], start=(k == 0), stop=(k == NKC - 1),
                )
            nc.vector.tensor_copy(v_sb[:, r, :, DK:2 * DK], vp[:])

        # ---- attention ----
        def attn_j(hpair, pts, J):
            width = min(640, T - J * 128)
            wA = min(512, width)
            wB = width - wA
            for part in range(2):           # row-group-alternating A then B
                for h in hpair:
                    hp = slice((h % 2) * 64, (h % 2) * 64 + 64)
                    hc = h // 2
                    if part == 0:
                        pt = pt_pool.tile([128, 640], F16, tag="pt",
                                          name=f"pt_h{h}_J{J}")
                        pts[h][J] = pt
                        s = ps_s.tile([128, 640], F32, tag="s",
                                      name=f"s_h{h}_J{J}")
                        pts[h][(J, "s")] = s
                        nc.tensor.matmul(
                            s[:, 0:wA], k_sb[hp, hc, J * 128:(J + 1) * 128],
                            q_sb[hp, hc, J * 128:J * 128 + wA],
                            start=True, stop=True,
                        )
                    else:
                        s = pts[h].pop((J, "s"))
                        pt = pts[h][J]
                        if wB > 0:
                            nc.tensor.matmul(
                                s[:, 512:512 + wB],
                                k_sb[hp, hc, J * 128:(J + 1) * 128],
                                q_sb[hp, hc, J * 128 + 512:J * 128 + width],
                                start=True, stop=True,
                            )
                        nc.scalar.activation(
                            pt[:, 0:width], s[:, 0:width],
                            mybir.ActivationFunctionType.Exp, scale=0.125,
                        )
                        nc.gpsimd.tensor_mul(pt[:, 0:128], pt[:, 0:128],
                                             mask_lo[:])
                        if width == 640:
                            nc.gpsimd.tensor_mul(pt[:, 512:640],
                                                 pt[:, 512:640], mask_hi[:])

        norm_state = {}

        def attn_pv(hpair, pts, g):
            g0 = 512 * g
            pvs = {}
            for h in hpair:
                pv = ps_pv.tile([128, 512], F32, tag="pv", name=f"pv_h{h}_g{g}")
                pvs[h] = pv
                jps = []
                for Jp in range(max(0, 4 * g - 4), 4 * g + 4):
                    wJp = min(640, T - Jp * 128)
                    lo = max(Jp * 128, g0)
                    hi = min(Jp * 128 + wJp, g0 + 512)
                    if hi > lo:
                        jps.append((Jp, lo, hi))
                # start=True lazily zeroes the whole psum bank; a full-width
                # contribution must come first
                jps.sort(key=lambda t: -(t[2] - t[1]))
                assert jps[0][2] - jps[0][1] == 512
                for idx, (Jp, lo, hi) in enumerate(jps):
                    nc.tensor.matmul(
                        pv[:, lo - g0:hi - g0],
                        v_sb[:, Jp, h, :],
                        pts[h][Jp][:, lo - Jp * 128:hi - Jp * 128],
                        start=(idx == 0), stop=(idx == len(jps) - 1),
                    )
                for Jp in range(max(0, 4 * g - 4), 4 * g):
                    pts[h].pop(Jp, None)
            norm_state[(hpair, g)] = pvs

        def attn_norm(hpair, g):
            # deferred ~1 step after attn_pv.  MUST be emitted before the pv
            # pool rotates into these tiles again (the osb multiply is the
            # psum tile's last reader).
            g0 = 512 * g
            pvs = norm_state.pop((hpair, g))
            for h in hpair:
                rcp = nrm_pool.tile([64, 512], F32, tag="rcp",
                                    name=f"rcp_h{h}_g{g}")
                nc.vector.reciprocal_approx_fast(rcp[:], pvs[h][0:64, :])
                hp = slice((h % 2) * 64, (h % 2) * 64 + 64)
                nc.vector.tensor_mul(
                    osb[hp, h // 2, g0:g0 + 512], pvs[h][64:128, :], rcp[:],
                )

        def o_proj(qbs, alt_pool=False):
            # runs 1-2 steps after both pairs normalized the group; spread
            # across non-group steps so the mi-ring and the psum->SBUF copy
            # engines aren't slammed at group boundaries.  At the very tail
            # the S psum pool is retired, so its 4 banks double the po ring.
            for qb in qbs:
                so = stage_pool.tile([128, 1024], F16, tag="stage",
                                     name=f"so{qb}")
                for nh in range(2):
                    if alt_pool and nh == 1:
                        po = ps_s.tile([128, 512], F32, tag="s",
                                       name=f"po{qb}_{nh}")
                    else:
                        po = ps_mi.tile([128, 512], F32, tag="mi",
                                        name=f"po{qb}_{nh}")
                    for c in range(2):
                        nc.tensor.matmul(
                            po[:], osb[:, c, qb * 128:(qb + 1) * 128],
                            wo_sb[:, c, nh * 512:(nh + 1) * 512],
                            start=(c == 0), stop=(c == 1),
                        )
                    if nh == 0:
                        nc.scalar.copy(so[:, 0:512], po[:])
                    else:
                        nc.vector.tensor_copy(so[:, 512:1024], po[:])
                nc.sync.dma_start(
                    out_d[qb * 128:(qb + 1) * 128, :], so[:, :])

        # ---- interleaved 18-step pipeline ----
        pt01 = {0: {}, 1: {}}
        pt23 = {2: {}, 3: {}}
        P01, P23 = (0, 1), (2, 3)
        PV01 = lambda g: attn_pv(P01, pt01, g)
        PV23 = lambda g: attn_pv(P23, pt23, g)
        N01 = lambda g: attn_norm(P01, g)
        N23 = lambda g: attn_norm(P23, g)
        XT3 = lambda: xt_dma(3, engs=(nc.sync,))
        OP = lambda qbs: o_proj(qbs)

        u_q(0, 0); u_k(0, 0); u_q(1, 0); u_k(1, 0)

        FILL = {
            0:  ([lambda: u_q(0, 1)],  [lambda: u_k(0, 1)]),
            1:  ([lambda: u_q(1, 1)],  [lambda: u_q(2, 0), lambda: u_v(0)]),
            2:  ([lambda: u_v(1)],     [lambda: u_v(2), lambda: u_v(3)]),
            3:  ([lambda: u_q(2, 1)],  [lambda: PV01(0), XT3]),
            4:  ([lambda: u_k(1, 1)],  [lambda: N01(0), lambda: PV23(0), lambda: u_v(4)]),
            5:  ([lambda: u_k(2, 0)],  [lambda: N23(0), lambda: OP([0, 1]), lambda: u_v(5)]),
            6:  ([lambda: u_k(2, 1)],  [lambda: OP([2, 3]), lambda: u_v(6)]),
            7:  ([lambda: u_v(7)],     [lambda: PV01(1), lambda: u_q(3, 0)]),
            8:  ([lambda: u_q(3, 1)],  [lambda: N01(1), lambda: PV23(1), lambda: u_v(8)]),
            9:  ([lambda: u_v(9)],     [lambda: N23(1), lambda: OP([4, 5])]),
            10: ([lambda: u_k(3, 0)],  [lambda: OP([6, 7]), lambda: u_v(10)]),
            11: ([lambda: u_v(11)],    [lambda: PV01(2)]),
            12: ([lambda: u_k(3, 1)],  [lambda: N01(2), lambda: PV23(2), lambda: u_v(12)]),
            13: ([lambda: u_v(13)],    [lambda: N23(2), lambda: OP([8, 9])]),
            14: ([lambda: u_v(14)],    [lambda: OP([10, 11]), lambda: u_v(15)]),
        }

        for s in range(15):
            pre, post = FILL[s]
            if s < NJ:
                attn_j(P01, pt01, s)
            for f in pre:
                f()
            if s >= 1:
                attn_j(P23, pt23, s - 1)
            if s == 14:
                attn_j(P23, pt23, 14)
            for f in post:
                f()
        # compressed tail: both pairs' last S, PV, norms and the final
        # output projections in one step.  ACT is idle here (no exps left),
        # so it takes the denominator copies off the DVE critical path.
        attn_j(P01, pt01, 15)
        attn_j(P23, pt23, 15)
        attn_pv(P01, pt01, 3)
        attn_norm(P01, 3)
        attn_pv(P23, pt23, 3)
        attn_norm(P23, 3)
        o_proj([12, 13, 14, 15], alt_pool=True)


def _build():
    if "nc" in _NC_CACHE:
        return _NC_CACHE["nc"]
    nc = bacc.Bacc("TRN2", debug=False)
    with tile.TileContext(nc) as tc:
        _emit(tc)
    nc.compile()
    _NC_CACHE["nc"] = nc
    return nc


def _shard_inputs(x, Wq, bq, Wk, Wv, Wo):
    idx = np.arange(128)
    mlo = (idx[None, :] >= idx[:, None]).astype(np.float16)  # c >= p
    mhi = (idx[None, :] < idx[:, None]).astype(np.float16)   # c < p
    in_maps = []
    for b in range(2):
        xT = np.ascontiguousarray(x[b].T).astype(np.float16)
        for hg in range(4):
            cols = slice(hg * HCOLS, (hg + 1) * HCOLS)
            in_maps.append({
                "xT": xT,
                "wq": np.ascontiguousarray(Wq[:, cols]).astype(np.float16),
                "wk": np.ascontiguousarray(Wk[:, cols]).astype(np.float16),
                "wv": np.ascontiguousarray(Wv[:, cols]).astype(np.float16),
                "wo": np.ascontiguousarray(Wo[cols, :]).astype(np.float16),
                "bqp": np.ascontiguousarray(bq[cols].reshape(2, 128).T),
                "mlo": mlo, "mhi": mhi,
            })
    return in_maps


def kernel(x, Wq, bq, Wk, bk, Wv, bv, Wo, bo, _trace=False, _tmpdir=None):
    x = np.asarray(x, dtype=np.float32)
    Wq = np.asarray(Wq, dtype=np.float32)
    Wk = np.asarray(Wk, dtype=np.float32)
    Wv = np.asarray(Wv, dtype=np.float32)
    Wo = np.asarray(Wo, dtype=np.float32)
    bq = np.asarray(bq, dtype=np.float32)
    bv = np.asarray(bv, dtype=np.float32)
    bo = np.asarray(bo, dtype=np.float32)

    nc = _build()
    in_maps = _shard_inputs(x, Wq, bq, Wk, Wv, Wo)
    res = run_bass_kernel_spmd(
        nc, in_maps, core_ids=list(range(8)), trace=_trace, tmpdir=_tmpdir,
    )
    host_bias = (bv @ Wo + bo).astype(np.float32)
    out = np.zeros((2, T, D), dtype=np.float32)
    for b in range(2):
        acc = res.results[b * 4]["out"].astype(np.float32).copy()
        for hg in range(1, 4):
            acc += res.results[b * 4 + hg]["out"]
        out[b] = acc + host_bias
    kernel._last_results = res
    return out
